# revision 4
# baseline (speedup 1.0000x reference)
"""Trainium2 Bass kernel for a dense transformer block (B=2, S=2048, D=2048,
H=16, head_dim=128, FF=8192, fp32 I/O), SPMD over 8 NeuronCores.

Sharding: data-parallel over tokens. Core c owns 512 tokens of batch b=c//4
(cores 0-3 -> batch 0, cores 4-7 -> batch 1). Attention needs all keys/values
of the batch, so K^T and V (bf16) are AllGather'd within each 4-core group.

Layout: activations live feature-major ("transposed", [D, tokens]) on chip so
every GEMM contracts along the partition axis with weights in natural layout.
The host transposes x in / y out (free). LayerNorm stats and softmax
denominators are partition-axis reductions done with ones-matmuls on the PE.

dtypes: weight GEMMs run float32r (full PE rate at N=512, ~13-bit rounding);
attention internals (scores, exp, PV, o_proj) and fc2 run bf16 with fp32 PSUM
accumulation.
"""
import sys

for _p in (
    "/root/.axon_site",
    "/root/.axon_site/_ro/trn_rl_repo",
    "/root/.axon_site/_ro/pypackages",
):
    if _p not in sys.path:
        sys.path.insert(0, _p)

import numpy as np

import concourse.bacc as bacc
import concourse.tile as tile
import concourse.mybir as mybir
from concourse import bass_utils
from concourse.alu_op_type import AluOpType
from concourse.bass_interp import get_hw_module

B, S, D = 2, 2048, 2048
H, HD, FF = 16, 128, 8192
N_CORES = 8
GROUPS = [[0, 1, 2, 3], [4, 5, 6, 7]]
GSIZE = 4
T = (B * S) // N_CORES  # 512 tokens per core
NCH = D // 128  # 16 feature chunks
FFCH = FF // 128  # 64 ff chunks
F32 = mybir.dt.float32
F32R = mybir.dt.float32r
BF16 = mybir.dt.bfloat16
AF = mybir.ActivationFunctionType
OP = AluOpType
SM_SCALE = 1.0 / float(np.sqrt(HD))


def _layernorm(nc, tc, src, dst, g_s, b_s, ones_r, eps_t, name):
    """dst[:, c, :] = LN(src)[:, c, :] over the feature (partition x chunk)
    axis. src/dst are [128, NCH, T] tiles, src f32r-typed, dst f32r."""
    with (
        tc.tile_pool(name=f"{name}_p", bufs=1) as lp,
        tc.tile_pool(name=f"{name}_s", bufs=3) as ls,
        tc.tile_pool(name=f"{name}_ps", bufs=1, space="PSUM") as lps,
    ):
        mu_ps = lps.tile([128, T], F32, tag="mu")
        for c in range(NCH):
            nc.tensor.matmul(
                mu_ps[:], ones_r[:], src[:, c, :],
                start=(c == 0), stop=(c == NCH - 1),
            )
        mu = lp.tile([128, T], F32)
        nc.scalar.activation(mu[:], mu_ps[:], AF.Copy, scale=1.0 / D)

        xc = lp.tile([128, NCH, T], F32)
        var_ps = lps.tile([128, T], F32, tag="var")
        for c in range(NCH):
            nc.vector.tensor_tensor(
                xc[:, c, :], src[:, c, :].bitcast(F32), mu[:], OP.subtract
            )
            sq = ls.tile([128, T], F32R, tag="sq")
            nc.scalar.activation(sq[:], xc[:, c, :], AF.Square)
            nc.tensor.matmul(
                var_ps[:], ones_r[:], sq[:],
                start=(c == 0), stop=(c == NCH - 1),
            )
        sd = lp.tile([128, T], F32)
        nc.scalar.activation(sd[:], var_ps[:], AF.Sqrt, bias=eps_t[:], scale=1.0 / D)
        rsq = lp.tile([128, T], F32)
        nc.vector.reciprocal_approx_fast(rsq[:], sd[:])

        for c in range(NCH):
            tmp = ls.tile([128, T], F32, tag="lnt")
            nc.vector.tensor_tensor(tmp[:], xc[:, c, :], rsq[:], OP.mult)
            nc.vector.tensor_scalar(
                dst[:, c, :], tmp[:],
                g_s[:, c:c + 1], b_s[:, c:c + 1],
                OP.mult, OP.add,
            )


def build():
    nc = bacc.Bacc("TRN2", target_bir_lowering=False, debug=False,
                   num_devices=N_CORES)

    xT_d = nc.dram_tensor("xT", [D, T], F32, kind="ExternalInput")
    wq_d = nc.dram_tensor("wq", [D, D], F32, kind="ExternalInput")
    wk_d = nc.dram_tensor("wk", [D, D], F32, kind="ExternalInput")
    wv_d = nc.dram_tensor("wv", [D, D], F32, kind="ExternalInput")
    wo_d = nc.dram_tensor("wo", [D, D], F32, kind="ExternalInput")
    w1_d = nc.dram_tensor("w1", [D, FF], F32, kind="ExternalInput")
    w2_d = nc.dram_tensor("w2", [FF, D], F32, kind="ExternalInput")
    b1_d = nc.dram_tensor("b1r", [128, FFCH], F32, kind="ExternalInput")
    b2_d = nc.dram_tensor("b2r", [128, NCH], F32, kind="ExternalInput")
    g1_d = nc.dram_tensor("g1r", [128, NCH], F32, kind="ExternalInput")
    be1_d = nc.dram_tensor("be1r", [128, NCH], F32, kind="ExternalInput")
    g2_d = nc.dram_tensor("g2r", [128, NCH], F32, kind="ExternalInput")
    be2_d = nc.dram_tensor("be2r", [128, NCH], F32, kind="ExternalInput")
    yT_d = nc.dram_tensor("yT", [D, T], F32, kind="ExternalOutput")

    with tile.TileContext(nc) as tc:
        with (
            tc.tile_pool(name="cst", bufs=1) as cst,
            tc.tile_pool(name="resid", bufs=1) as resid,
            tc.tile_pool(name="dram", bufs=1, space="DRAM") as dram,
        ):
            ones_r = cst.tile([128, 128], F32R)
            nc.vector.memset(ones_r[:].bitcast(F32), 1.0)
            ones16 = cst.tile([128, 128], BF16)
            nc.vector.memset(ones16[:], 1.0)
            eps_t = cst.tile([128, 1], F32)
            nc.vector.memset(eps_t[:], 1e-5)
            g1_s = cst.tile([128, NCH], F32)
            be1_s = cst.tile([128, NCH], F32)
            g2_s = cst.tile([128, NCH], F32)
            be2_s = cst.tile([128, NCH], F32)
            b1_s = cst.tile([128, FFCH], F32)
            b2_s = cst.tile([128, NCH], F32)
            nc.sync.dma_start(g1_s[:], g1_d.ap())
            nc.sync.dma_start(be1_s[:], be1_d.ap())
            nc.sync.dma_start(g2_s[:], g2_d.ap())
            nc.sync.dma_start(be2_s[:], be2_d.ap())
            nc.sync.dma_start(b1_s[:], b1_d.ap())
            nc.sync.dma_start(b2_s[:], b2_d.ap())

            xTs = resid.tile([128, NCH, T], F32R)
            for c in range(NCH):
                nc.sync.dma_start(
                    xTs[:, c, :],
                    xT_d.ap()[c * 128:(c + 1) * 128, :].bitcast(F32R),
                )
            x2Ts = resid.tile([128, NCH, T], F32R)

            kin = dram.tile([D, T], BF16)
            vin = dram.tile([T, D], BF16)
            kout = dram.tile([GSIZE * D, T], BF16)
            vout = dram.tile([GSIZE * T, D], BF16)

            with tc.tile_pool(name="attnres", bufs=1) as ares:
                qT = ares.tile([128, H, T], BF16)  # Q^T; per-head slot reused for O^T

                with tc.tile_pool(name="p1", bufs=1) as p1:
                    hT = p1.tile([128, NCH, T], F32R)
                    _layernorm(nc, tc, xTs, hT, g1_s, be1_s, ones_r, eps_t, "ln1")

                    with (
                        tc.tile_pool(name="qkvs", bufs=3) as qs,
                        tc.tile_pool(name="qkvstg", bufs=4) as stg,
                        tc.tile_pool(name="qkvps", bufs=2, space="PSUM") as qps,
                    ):
                        # K^T = wk.T @ h^T   (feeds the AllGather first)
                        for blk in range(4):
                            kps = qps.tile([128, 4, T], F32, tag="qkv")
                            for c in range(NCH):
                                wt = qs.tile([128, 512], F32R, tag="w")
                                nc.sync.dma_start(
                                    wt[:],
                                    wk_d.ap()[c * 128:(c + 1) * 128,
                                              blk * 512:(blk + 1) * 512].bitcast(F32R),
                                )
                                for q in range(4):
                                    nc.tensor.matmul(
                                        kps[:, q, :], wt[:, q * 128:(q + 1) * 128],
                                        hT[:, c, :],
                                        start=(c == 0), stop=(c == NCH - 1),
                                    )
                            for q in range(4):
                                ks = stg.tile([128, T], BF16, tag="kstg")
                                nc.scalar.activation(ks[:], kps[:, q, :], AF.Copy)
                                dk = blk * 4 + q
                                nc.sync.dma_start(
                                    kin[dk * 128:(dk + 1) * 128, :], ks[:]
                                )
                        # V = h @ wv  (natural layout: lhsT = h^T chunk)
                        for blk in range(4):  # dv block of 512
                            vps = qps.tile([128, 4, T], F32, tag="qkv")
                            for c in range(NCH):
                                wt = qs.tile([128, 512], F32R, tag="w")
                                nc.sync.dma_start(
                                    wt[:],
                                    wv_d.ap()[c * 128:(c + 1) * 128,
                                              blk * 512:(blk + 1) * 512].bitcast(F32R),
                                )
                                for t_ in range(4):
                                    nc.tensor.matmul(
                                        vps[:, t_, :],
                                        hT[:, c, t_ * 128:(t_ + 1) * 128],
                                        wt[:],
                                        start=(c == 0), stop=(c == NCH - 1),
                                    )
                            for t_ in range(4):
                                vs = stg.tile([128, 512], BF16, tag="vstg")
                                nc.scalar.activation(vs[:], vps[:, t_, :], AF.Copy)
                                nc.sync.dma_start(
                                    vin[t_ * 128:(t_ + 1) * 128,
                                        blk * 512:(blk + 1) * 512],
                                    vs[:],
                                )

                        nc.gpsimd.collective_compute(
                            "AllGather",
                            OP.bypass,
                            replica_groups=GROUPS,
                            ins=[kin.opt()],
                            outs=[kout.opt()],
                        )
                        nc.gpsimd.collective_compute(
                            "AllGather",
                            OP.bypass,
                            replica_groups=GROUPS,
                            ins=[vin.opt()],
                            outs=[vout.opt()],
                        )

                        # Q^T (overlaps the collective)
                        for blk in range(4):
                            qph = qps.tile([128, 4, T], F32, tag="qkv")
                            for c in range(NCH):
                                wt = qs.tile([128, 512], F32R, tag="w")
                                nc.sync.dma_start(
                                    wt[:],
                                    wq_d.ap()[c * 128:(c + 1) * 128,
                                              blk * 512:(blk + 1) * 512].bitcast(F32R),
                                )
                                for q in range(4):
                                    nc.tensor.matmul(
                                        qph[:, q, :], wt[:, q * 128:(q + 1) * 128],
                                        hT[:, c, :],
                                        start=(c == 0), stop=(c == NCH - 1),
                                    )
                            for q in range(4):
                                nc.scalar.activation(
                                    qT[:, blk * 4 + q, :], qph[:, q, :], AF.Copy
                                )

                # attention
                with (
                    tc.tile_pool(name="attnv", bufs=1) as avp,
                    tc.tile_pool(name="atts", bufs=2) as ats,
                    tc.tile_pool(name="attes", bufs=3) as aes,
                    tc.tile_pool(name="attps", bufs=1, space="PSUM") as aps,
                    tc.tile_pool(name="attps2", bufs=2, space="PSUM") as aps2,
                ):
                    vres = avp.tile([128, GSIZE * T // 128, D], BF16)
                    for j in range(GSIZE * T // 128):
                        nc.sync.dma_start(
                            vres[:, j, :], vout[j * 128:(j + 1) * 128, :]
                        )
                    for h in range(H):
                        kT = ats.tile([128, GSIZE, T], BF16, tag="kT")
                        for r in range(GSIZE):
                            nc.sync.dma_start(
                                kT[:, r, :],
                                kout[r * D + h * 128: r * D + (h + 1) * 128, :],
                            )
                        pv_ps = aps2.tile([128, T], F32, tag="pv")
                        den_ps = aps2.tile([128, T], F32, tag="den")
                        for jb in range(4):
                            s_ps = aps.tile([128, 4, T], F32, tag="s")
                            for jj in range(4):
                                j = jb * 4 + jj
                                nc.tensor.matmul(
                                    s_ps[:, jj, :],
                                    kT[:, j // 4, (j % 4) * 128:(j % 4 + 1) * 128],
                                    qT[:, h, :],
                                    start=True, stop=True,
                                )
                            exps = aes.tile([128, 4, T], BF16, tag="exp")
                            nc.scalar.activation(
                                exps[:], s_ps[:], AF.Exp, scale=SM_SCALE
                            )
                            for jj in range(4):
                                j = jb * 4 + jj
                                nc.tensor.matmul(
                                    pv_ps[:],
                                    vres[:, j, h * 128:(h + 1) * 128],
                                    exps[:, jj, :],
                                    start=(j == 0), stop=(j == GSIZE * 4 - 1),
                                )
                                nc.tensor.matmul(
                                    den_ps[:], ones16[:], exps[:, jj, :],
                                    start=(j == 0), stop=(j == GSIZE * 4 - 1),
                                )
                        rec = aes.tile([128, T], F32, tag="rec")
                        nc.vector.reciprocal_approx_fast(rec[:], den_ps[:])
                        # overwrite Q^T slot with O^T (Q^T[h] is dead now)
                        nc.vector.tensor_tensor(
                            qT[:, h, :], pv_ps[:], rec[:], OP.mult
                        )

                # o_proj + residual -> x2T
                with (
                    tc.tile_pool(name="ops", bufs=3) as osp,
                    tc.tile_pool(name="opps", bufs=2, space="PSUM") as ops_ps,
                ):
                    for blk in range(4):
                        o_ps = ops_ps.tile([128, 4, T], F32, tag="o")
                        for h in range(H):
                            wt = osp.tile([128, 512], BF16, tag="wo")
                            nc.gpsimd.dma_start(
                                wt[:],
                                wo_d.ap()[h * 128:(h + 1) * 128,
                                          blk * 512:(blk + 1) * 512],
                            )
                            for q in range(4):
                                nc.tensor.matmul(
                                    o_ps[:, q, :], wt[:, q * 128:(q + 1) * 128],
                                    qT[:, h, :],
                                    start=(h == 0), stop=(h == H - 1),
                                )
                        for q in range(4):
                            dc = blk * 4 + q
                            nc.vector.tensor_tensor(
                                x2Ts[:, dc, :], o_ps[:, q, :],
                                xTs[:, dc, :].bitcast(F32), OP.add,
                            )

            # FFN
            with tc.tile_pool(name="ffnres", bufs=1) as fres:
                h2T = fres.tile([128, NCH, T], F32R)
                _layernorm(nc, tc, x2Ts, h2T, g2_s, be2_s, ones_r, eps_t, "ln2")

                with (
                    tc.tile_pool(name="gpool", bufs=1) as gp,
                    tc.tile_pool(name="fcs", bufs=3) as fs,
                    tc.tile_pool(name="fco", bufs=3) as fo,
                    tc.tile_pool(name="fc1ps", bufs=1, space="PSUM") as f1ps,
                    tc.tile_pool(name="fc2ps", bufs=1, space="PSUM") as f2ps,
                ):
                    gres = gp.tile([128, FFCH, T], BF16)
                    for fb in range(16):
                        a_ps = f1ps.tile([128, 4, T], F32, tag="a")
                        for c in range(NCH):
                            wt = fs.tile([128, 512], F32R, tag="w1")
                            nc.sync.dma_start(
                                wt[:],
                                w1_d.ap()[c * 128:(c + 1) * 128,
                                          fb * 512:(fb + 1) * 512].bitcast(F32R),
                            )
                            for q in range(4):
                                nc.tensor.matmul(
                                    a_ps[:, q, :], wt[:, q * 128:(q + 1) * 128],
                                    h2T[:, c, :],
                                    start=(c == 0), stop=(c == NCH - 1),
                                )
                        for q in range(4):
                            ffc = fb * 4 + q
                            nc.scalar.activation(
                                gres[:, ffc, :], a_ps[:, q, :], AF.Gelu,
                                bias=b1_s[:, ffc:ffc + 1],
                            )
                    for db in range(4):
                        y_ps = f2ps.tile([128, 4, T], F32, tag="y")
                        for f in range(FFCH):
                            wt = fs.tile([128, 512], BF16, tag="w2")
                            nc.gpsimd.dma_start(
                                wt[:],
                                w2_d.ap()[f * 128:(f + 1) * 128,
                                          db * 512:(db + 1) * 512],
                            )
                            for q in range(4):
                                nc.tensor.matmul(
                                    y_ps[:, q, :], wt[:, q * 128:(q + 1) * 128],
                                    gres[:, f, :],
                                    start=(f == 0), stop=(f == FFCH - 1),
                                )
                        for q in range(4):
                            dc = db * 4 + q
                            yt = fo.tile([128, T], F32, tag="yt")
                            nc.vector.scalar_tensor_tensor(
                                yt[:], y_ps[:, q, :], b2_s[:, dc:dc + 1],
                                x2Ts[:, dc, :].bitcast(F32),
                                OP.add, OP.add,
                            )
                            nc.sync.dma_start(
                                yT_d.ap()[dc * 128:(dc + 1) * 128, :], yt[:]
                            )

    nc.compile()
    return nc


_NC_CACHE = None


def _get_nc():
    global _NC_CACHE
    if _NC_CACHE is None:
        m = build()
        m.m = get_hw_module(m.m)
        _NC_CACHE = m
    return _NC_CACHE


def _make_in_maps(x, wq, wk, wv, wo, w1, b1, w2, b2, g1, be1, g2, be2):
    f = lambda a: np.ascontiguousarray(np.asarray(a, dtype=np.float32))
    x = f(x)
    shared = {
        "wq": f(wq), "wk": f(wk), "wv": f(wv), "wo": f(wo),
        "w1": f(w1), "w2": f(w2),
        "b1r": np.ascontiguousarray(f(b1).reshape(FFCH, 128).T),
        "b2r": np.ascontiguousarray(f(b2).reshape(NCH, 128).T),
        "g1r": np.ascontiguousarray(f(g1).reshape(NCH, 128).T),
        "be1r": np.ascontiguousarray(f(be1).reshape(NCH, 128).T),
        "g2r": np.ascontiguousarray(f(g2).reshape(NCH, 128).T),
        "be2r": np.ascontiguousarray(f(be2).reshape(NCH, 128).T),
    }
    in_maps = []
    for c in range(N_CORES):
        b, t0 = c // GSIZE, (c % GSIZE) * T
        m = dict(shared)
        m["xT"] = np.ascontiguousarray(x[b, t0:t0 + T, :].T)
        in_maps.append(m)
    return in_maps


def _assemble(results):
    y = np.empty((B, S, D), dtype=np.float32)
    for c in range(N_CORES):
        b, t0 = c // GSIZE, (c % GSIZE) * T
        y[b, t0:t0 + T, :] = results[c]["yT"].T
    return y


def run(inputs, trace=False, trace_cores=None):
    nc = _get_nc()
    in_maps = _make_in_maps(**inputs)
    res = bass_utils.run_bass_kernel_spmd(
        nc, in_maps, core_ids=list(range(N_CORES)),
        trace=trace, trace_cores=trace_cores,
    )
    return _assemble(res.results), res


def kernel(**inputs):
    y, _ = run(inputs, trace=False)
    return y


# revision 6
# speedup vs baseline: 1.2769x; 1.2769x over previous
"""Trainium2 Bass kernel for a dense transformer block (B=2, S=2048, D=2048,
H=16, head_dim=128, FF=8192, fp32 I/O), SPMD over 8 NeuronCores.

Sharding: data-parallel over tokens. Core c owns 512 tokens of batch b=c//4
(cores 0-3 -> batch 0, cores 4-7 -> batch 1). Attention needs all keys/values
of the batch, so K^T and V (bf16) are AllGather'd within each 4-core group.

Layout: activations live feature-major ("transposed", [D, tokens]) on chip so
every GEMM contracts along the partition axis with weights in natural layout.
The host transposes x in / y out (free). LayerNorm stats and softmax
denominators are partition-axis reductions done with ones-matmuls on the PE.

dtypes: weight GEMMs run float32r (full PE rate at N=512, ~13-bit rounding);
attention internals (scores, exp, PV) run bf16; o_proj and fc2 take bf16
weights pre-cast on the host. All accumulation is fp32 in PSUM.

Weight streams use [128, 1024] tiles with deep (bufs=6) prefetch: shallow
prefetch stalls the PE on DMA latency, which also re-throttles the PE clock
(HAM) to half rate.
"""
import sys

for _p in (
    "/root/.axon_site",
    "/root/.axon_site/_ro/trn_rl_repo",
    "/root/.axon_site/_ro/pypackages",
):
    if _p not in sys.path:
        sys.path.insert(0, _p)

import ml_dtypes
import numpy as np

import concourse.bacc as bacc
import concourse.tile as tile
import concourse.mybir as mybir
from concourse import bass_utils
from concourse.alu_op_type import AluOpType
from concourse.bass_interp import get_hw_module

B, S, D = 2, 2048, 2048
H, HD, FF = 16, 128, 8192
N_CORES = 8
GROUPS = [[0, 1, 2, 3], [4, 5, 6, 7]]
GSIZE = 4
T = (B * S) // N_CORES  # 512 tokens per core
NCH = D // 128  # 16 feature chunks
FFCH = FF // 128  # 64 ff chunks
F32 = mybir.dt.float32
F32R = mybir.dt.float32r
BF16 = mybir.dt.bfloat16
AF = mybir.ActivationFunctionType
OP = AluOpType
SM_SCALE = 1.0 / float(np.sqrt(HD))
WBUFS = 6  # weight-stream prefetch depth


def _layernorm(nc, tc, src, dst, g_s, b_s, ones_r, eps_t, name):
    """dst[:, c, :] = LN(src)[:, c, :] over the feature (partition x chunk)
    axis. src/dst are [128, NCH, T] tiles, f32r-typed."""
    with (
        tc.tile_pool(name=f"{name}_p", bufs=1) as lp,
        tc.tile_pool(name=f"{name}_s", bufs=3) as ls,
        tc.tile_pool(name=f"{name}_ps", bufs=1, space="PSUM") as lps,
    ):
        mu_ps = lps.tile([128, T], F32, tag="mu")
        for c in range(NCH):
            nc.tensor.matmul(
                mu_ps[:], ones_r[:], src[:, c, :],
                start=(c == 0), stop=(c == NCH - 1),
            )
        mu = lp.tile([128, T], F32)
        nc.scalar.activation(mu[:], mu_ps[:], AF.Copy, scale=1.0 / D)

        xc = lp.tile([128, NCH, T], F32)
        var_ps = lps.tile([128, T], F32, tag="var")
        for c in range(NCH):
            nc.vector.tensor_tensor(
                xc[:, c, :], src[:, c, :].bitcast(F32), mu[:], OP.subtract
            )
            sq = ls.tile([128, T], F32R, tag="sq")
            nc.scalar.activation(sq[:], xc[:, c, :], AF.Square)
            nc.tensor.matmul(
                var_ps[:], ones_r[:], sq[:],
                start=(c == 0), stop=(c == NCH - 1),
            )
        sd = lp.tile([128, T], F32)
        nc.scalar.activation(sd[:], var_ps[:], AF.Sqrt, bias=eps_t[:], scale=1.0 / D)
        rsq = lp.tile([128, T], F32)
        nc.vector.reciprocal_approx_fast(rsq[:], sd[:])

        for c in range(NCH):
            tmp = ls.tile([128, T], F32, tag="lnt")
            nc.vector.tensor_tensor(tmp[:], xc[:, c, :], rsq[:], OP.mult)
            nc.vector.tensor_scalar(
                dst[:, c, :], tmp[:],
                g_s[:, c:c + 1], b_s[:, c:c + 1],
                OP.mult, OP.add,
            )


def build():
    nc = bacc.Bacc("TRN2", target_bir_lowering=False, debug=False,
                   num_devices=N_CORES)

    xT_d = nc.dram_tensor("xT", [D, T], F32, kind="ExternalInput")
    wq_d = nc.dram_tensor("wq", [D, D], F32, kind="ExternalInput")
    wk_d = nc.dram_tensor("wk", [D, D], F32, kind="ExternalInput")
    wv_d = nc.dram_tensor("wv", [D, D], F32, kind="ExternalInput")
    wo_d = nc.dram_tensor("wo16", [D, D], BF16, kind="ExternalInput")
    w1_d = nc.dram_tensor("w1", [D, FF], F32, kind="ExternalInput")
    w2_d = nc.dram_tensor("w216", [FF, D], BF16, kind="ExternalInput")
    b1_d = nc.dram_tensor("b1r", [128, FFCH], F32, kind="ExternalInput")
    b2_d = nc.dram_tensor("b2r", [128, NCH], F32, kind="ExternalInput")
    g1_d = nc.dram_tensor("g1r", [128, NCH], F32, kind="ExternalInput")
    be1_d = nc.dram_tensor("be1r", [128, NCH], F32, kind="ExternalInput")
    g2_d = nc.dram_tensor("g2r", [128, NCH], F32, kind="ExternalInput")
    be2_d = nc.dram_tensor("be2r", [128, NCH], F32, kind="ExternalInput")
    yT_d = nc.dram_tensor("yT", [D, T], F32, kind="ExternalOutput")

    with tile.TileContext(nc) as tc:
        with (
            tc.tile_pool(name="cst", bufs=1) as cst,
            tc.tile_pool(name="resid", bufs=1) as resid,
            tc.tile_pool(name="dram", bufs=1, space="DRAM") as dram,
        ):
            ones_r = cst.tile([128, 128], F32R)
            nc.vector.memset(ones_r[:].bitcast(F32), 1.0)
            ones16 = cst.tile([128, 128], BF16)
            nc.vector.memset(ones16[:], 1.0)
            eps_t = cst.tile([128, 1], F32)
            nc.vector.memset(eps_t[:], 1e-5)
            g1_s = cst.tile([128, NCH], F32)
            be1_s = cst.tile([128, NCH], F32)
            g2_s = cst.tile([128, NCH], F32)
            be2_s = cst.tile([128, NCH], F32)
            b1_s = cst.tile([128, FFCH], F32)
            b2_s = cst.tile([128, NCH], F32)
            nc.sync.dma_start(g1_s[:], g1_d.ap())
            nc.sync.dma_start(be1_s[:], be1_d.ap())
            nc.sync.dma_start(g2_s[:], g2_d.ap())
            nc.sync.dma_start(be2_s[:], be2_d.ap())
            nc.sync.dma_start(b1_s[:], b1_d.ap())
            nc.sync.dma_start(b2_s[:], b2_d.ap())

            xTs = resid.tile([128, NCH, T], F32R)
            for c in range(NCH):
                nc.sync.dma_start(
                    xTs[:, c, :],
                    xT_d.ap()[c * 128:(c + 1) * 128, :].bitcast(F32R),
                )
            x2Ts = resid.tile([128, NCH, T], F32R)

            kin = dram.tile([D, T], BF16)
            vin = dram.tile([T, D], BF16)
            kout = dram.tile([GSIZE * D, T], BF16)
            vout = dram.tile([GSIZE * T, D], BF16)

            with tc.tile_pool(name="attnres", bufs=1) as ares:
                qT = ares.tile([128, H, T], BF16)  # Q^T; per-head slot reused for O^T

                with tc.tile_pool(name="p1", bufs=1) as p1:
                    hT = p1.tile([128, NCH, T], F32R)
                    _layernorm(nc, tc, xTs, hT, g1_s, be1_s, ones_r, eps_t, "ln1")

                    with (
                        tc.tile_pool(name="qkvs", bufs=WBUFS) as qs,
                        tc.tile_pool(name="qkvstg", bufs=4) as stg,
                        tc.tile_pool(name="qkvps", bufs=1, space="PSUM") as qps,
                    ):
                        # K^T = wk.T @ h^T   (feeds the AllGather first)
                        for blk in range(2):
                            kps = qps.tile([128, 8, T], F32, tag="qkv")
                            for c in range(NCH):
                                wt = qs.tile([128, 1024], F32R, tag="w")
                                nc.sync.dma_start(
                                    wt[:],
                                    wk_d.ap()[c * 128:(c + 1) * 128,
                                              blk * 1024:(blk + 1) * 1024].bitcast(F32R),
                                )
                                for q in range(8):
                                    nc.tensor.matmul(
                                        kps[:, q, :], wt[:, q * 128:(q + 1) * 128],
                                        hT[:, c, :],
                                        start=(c == 0), stop=(c == NCH - 1),
                                    )
                            for q in range(8):
                                ks = stg.tile([128, T], BF16, tag="kstg")
                                nc.scalar.activation(ks[:], kps[:, q, :], AF.Copy)
                                dk = blk * 8 + q
                                nc.sync.dma_start(
                                    kin[dk * 128:(dk + 1) * 128, :], ks[:]
                                )
                        # V = h @ wv  (natural layout: lhsT = h^T chunk)
                        for blk in range(2):
                            vps = qps.tile([128, 8, T], F32, tag="qkv")
                            for c in range(NCH):
                                wt = qs.tile([128, 1024], F32R, tag="w")
                                nc.sync.dma_start(
                                    wt[:],
                                    wv_d.ap()[c * 128:(c + 1) * 128,
                                              blk * 1024:(blk + 1) * 1024].bitcast(F32R),
                                )
                                for sub in range(2):
                                    for t_ in range(4):
                                        nc.tensor.matmul(
                                            vps[:, sub * 4 + t_, :],
                                            hT[:, c, t_ * 128:(t_ + 1) * 128],
                                            wt[:, sub * 512:(sub + 1) * 512],
                                            start=(c == 0), stop=(c == NCH - 1),
                                        )
                            for sub in range(2):
                                for t_ in range(4):
                                    vs = stg.tile([128, 512], BF16, tag="vstg")
                                    nc.scalar.activation(
                                        vs[:], vps[:, sub * 4 + t_, :], AF.Copy
                                    )
                                    dv = blk * 2 + sub
                                    nc.sync.dma_start(
                                        vin[t_ * 128:(t_ + 1) * 128,
                                            dv * 512:(dv + 1) * 512],
                                        vs[:],
                                    )

                        nc.gpsimd.collective_compute(
                            "AllGather",
                            OP.bypass,
                            replica_groups=GROUPS,
                            ins=[kin.opt()],
                            outs=[kout.opt()],
                        )
                        nc.gpsimd.collective_compute(
                            "AllGather",
                            OP.bypass,
                            replica_groups=GROUPS,
                            ins=[vin.opt()],
                            outs=[vout.opt()],
                        )
                        # Q^T (overlaps the collective)
                        for blk in range(2):
                            qph = qps.tile([128, 8, T], F32, tag="qkv")
                            for c in range(NCH):
                                wt = qs.tile([128, 1024], F32R, tag="w")
                                nc.sync.dma_start(
                                    wt[:],
                                    wq_d.ap()[c * 128:(c + 1) * 128,
                                              blk * 1024:(blk + 1) * 1024].bitcast(F32R),
                                )
                                for q in range(8):
                                    nc.tensor.matmul(
                                        qph[:, q, :], wt[:, q * 128:(q + 1) * 128],
                                        hT[:, c, :],
                                        start=(c == 0), stop=(c == NCH - 1),
                                    )
                            for q in range(8):
                                nc.scalar.activation(
                                    qT[:, blk * 8 + q, :], qph[:, q, :], AF.Copy
                                )

                # attention
                with (
                    tc.tile_pool(name="attnv", bufs=1) as avp,
                    tc.tile_pool(name="atts", bufs=3) as ats,
                    tc.tile_pool(name="attes", bufs=3) as aes,
                    tc.tile_pool(name="attps", bufs=1, space="PSUM") as aps,
                    tc.tile_pool(name="attps2", bufs=2, space="PSUM") as aps2,
                ):
                    vres = avp.tile([128, GSIZE * T // 128, D], BF16)
                    for j in range(GSIZE * T // 128):
                        nc.sync.dma_start(
                            vres[:, j, :], vout[j * 128:(j + 1) * 128, :]
                        )
                    for h in range(H):
                        kT = ats.tile([128, GSIZE, T], BF16, tag="kT")
                        for r in range(GSIZE):
                            nc.sync.dma_start(
                                kT[:, r, :],
                                kout[r * D + h * 128: r * D + (h + 1) * 128, :],
                            )
                        pv_ps = aps2.tile([128, T], F32, tag="pv")
                        den_ps = aps2.tile([128, T], F32, tag="den")
                        for jb in range(4):
                            s_ps = aps.tile([128, 4, T], F32, tag="s")
                            for jj in range(4):
                                j = jb * 4 + jj
                                nc.tensor.matmul(
                                    s_ps[:, jj, :],
                                    kT[:, j // 4, (j % 4) * 128:(j % 4 + 1) * 128],
                                    qT[:, h, :],
                                    start=True, stop=True,
                                )
                            exps = aes.tile([128, 4, T], BF16, tag="exp")
                            nc.scalar.activation(
                                exps[:], s_ps[:], AF.Exp, scale=SM_SCALE
                            )
                            for jj in range(4):
                                j = jb * 4 + jj
                                nc.tensor.matmul(
                                    pv_ps[:],
                                    vres[:, j, h * 128:(h + 1) * 128],
                                    exps[:, jj, :],
                                    start=(j == 0), stop=(j == GSIZE * 4 - 1),
                                )
                                nc.tensor.matmul(
                                    den_ps[:], ones16[:], exps[:, jj, :],
                                    start=(j == 0), stop=(j == GSIZE * 4 - 1),
                                )
                        rec = aes.tile([128, T], F32, tag="rec")
                        nc.vector.reciprocal_approx_fast(rec[:], den_ps[:])
                        # overwrite Q^T slot with O^T (Q^T[h] is dead now)
                        nc.vector.tensor_tensor(
                            qT[:, h, :], pv_ps[:], rec[:], OP.mult
                        )

                # o_proj + residual -> x2T
                with (
                    tc.tile_pool(name="ops", bufs=WBUFS) as osp,
                    tc.tile_pool(name="opps", bufs=1, space="PSUM") as ops_ps,
                ):
                    for blk in range(2):
                        o_ps = ops_ps.tile([128, 8, T], F32, tag="o")
                        for h in range(H):
                            wt = osp.tile([128, 1024], BF16, tag="wo")
                            nc.sync.dma_start(
                                wt[:],
                                wo_d.ap()[h * 128:(h + 1) * 128,
                                          blk * 1024:(blk + 1) * 1024],
                            )
                            for q in range(8):
                                nc.tensor.matmul(
                                    o_ps[:, q, :], wt[:, q * 128:(q + 1) * 128],
                                    qT[:, h, :],
                                    start=(h == 0), stop=(h == H - 1),
                                )
                        for q in range(8):
                            dc = blk * 8 + q
                            nc.vector.tensor_tensor(
                                x2Ts[:, dc, :], o_ps[:, q, :],
                                xTs[:, dc, :].bitcast(F32), OP.add,
                            )

            # FFN
            with tc.tile_pool(name="ffnres", bufs=1) as fres:
                h2T = fres.tile([128, NCH, T], F32R)
                _layernorm(nc, tc, x2Ts, h2T, g2_s, be2_s, ones_r, eps_t, "ln2")

                with tc.tile_pool(name="gpool", bufs=1) as gp:
                    gres = gp.tile([128, FFCH, T], BF16)
                    with (
                        tc.tile_pool(name="fc1s", bufs=WBUFS) as fs1,
                        tc.tile_pool(name="fc1ps", bufs=1, space="PSUM") as f1ps,
                    ):
                        for fb in range(8):
                            a_ps = f1ps.tile([128, 8, T], F32, tag="a")
                            for c in range(NCH):
                                wt = fs1.tile([128, 1024], F32R, tag="w1")
                                nc.sync.dma_start(
                                    wt[:],
                                    w1_d.ap()[c * 128:(c + 1) * 128,
                                              fb * 1024:(fb + 1) * 1024].bitcast(F32R),
                                )
                                for q in range(8):
                                    nc.tensor.matmul(
                                        a_ps[:, q, :], wt[:, q * 128:(q + 1) * 128],
                                        h2T[:, c, :],
                                        start=(c == 0), stop=(c == NCH - 1),
                                    )
                            for q in range(8):
                                ffc = fb * 8 + q
                                nc.scalar.activation(
                                    gres[:, ffc, :], a_ps[:, q, :], AF.Gelu,
                                    bias=b1_s[:, ffc:ffc + 1],
                                )
                    with (
                        tc.tile_pool(name="fc2s", bufs=WBUFS) as fs2,
                        tc.tile_pool(name="fco", bufs=3) as fo,
                        tc.tile_pool(name="fc2ps", bufs=1, space="PSUM") as f2ps,
                    ):
                        for db in range(2):
                            y_ps = f2ps.tile([128, 8, T], F32, tag="y")
                            for f in range(FFCH):
                                wt = fs2.tile([128, 1024], BF16, tag="w2")
                                nc.sync.dma_start(
                                    wt[:],
                                    w2_d.ap()[f * 128:(f + 1) * 128,
                                              db * 1024:(db + 1) * 1024],
                                )
                                for q in range(8):
                                    nc.tensor.matmul(
                                        y_ps[:, q, :], wt[:, q * 128:(q + 1) * 128],
                                        gres[:, f, :],
                                        start=(f == 0), stop=(f == FFCH - 1),
                                    )
                            for q in range(8):
                                dc = db * 8 + q
                                yt = fo.tile([128, T], F32, tag="yt")
                                nc.vector.scalar_tensor_tensor(
                                    yt[:], y_ps[:, q, :], b2_s[:, dc:dc + 1],
                                    x2Ts[:, dc, :].bitcast(F32),
                                    OP.add, OP.add,
                                )
                                nc.sync.dma_start(
                                    yT_d.ap()[dc * 128:(dc + 1) * 128, :], yt[:]
                                )

    nc.compile()
    return nc


_NC_CACHE = None


def _get_nc():
    global _NC_CACHE
    if _NC_CACHE is None:
        m = build()
        m.m = get_hw_module(m.m)
        _NC_CACHE = m
    return _NC_CACHE


def _make_in_maps(x, wq, wk, wv, wo, w1, b1, w2, b2, g1, be1, g2, be2):
    f = lambda a: np.ascontiguousarray(np.asarray(a, dtype=np.float32))
    f16 = lambda a: np.ascontiguousarray(
        np.asarray(a, dtype=np.float32).astype(ml_dtypes.bfloat16)
    )
    x = f(x)
    shared = {
        "wq": f(wq), "wk": f(wk), "wv": f(wv), "wo16": f16(wo),
        "w1": f(w1), "w216": f16(w2),
        "b1r": np.ascontiguousarray(f(b1).reshape(FFCH, 128).T),
        "b2r": np.ascontiguousarray(f(b2).reshape(NCH, 128).T),
        "g1r": np.ascontiguousarray(f(g1).reshape(NCH, 128).T),
        "be1r": np.ascontiguousarray(f(be1).reshape(NCH, 128).T),
        "g2r": np.ascontiguousarray(f(g2).reshape(NCH, 128).T),
        "be2r": np.ascontiguousarray(f(be2).reshape(NCH, 128).T),
    }
    in_maps = []
    for c in range(N_CORES):
        b, t0 = c // GSIZE, (c % GSIZE) * T
        m = dict(shared)
        m["xT"] = np.ascontiguousarray(x[b, t0:t0 + T, :].T)
        in_maps.append(m)
    return in_maps


def _assemble(results):
    y = np.empty((B, S, D), dtype=np.float32)
    for c in range(N_CORES):
        b, t0 = c // GSIZE, (c % GSIZE) * T
        y[b, t0:t0 + T, :] = results[c]["yT"].T
    return y


def run(inputs, trace=False, trace_cores=None):
    nc = _get_nc()
    in_maps = _make_in_maps(**inputs)
    res = bass_utils.run_bass_kernel_spmd(
        nc, in_maps, core_ids=list(range(N_CORES)),
        trace=trace, trace_cores=trace_cores,
    )
    return _assemble(res.results), res


def kernel(**inputs):
    y, _ = run(inputs, trace=False)
    return y


# revision 8
# speedup vs baseline: 1.3615x; 1.0663x over previous
"""Trainium2 Bass kernel for a dense transformer block (B=2, S=2048, D=2048,
H=16, head_dim=128, FF=8192, fp32 I/O), SPMD over 8 NeuronCores.

Sharding: data-parallel over tokens, batch-interleaved: core c owns tokens
[256c, 256c+256) of BOTH batches (512 tokens total). Attention needs all
keys/values of each batch, so K^T and V (bf16) are AllGather'd over all 8
cores — batch-interleaving keeps the gathered layout identical on every core
(no core-dependent addressing) and the 8-rank chip-wide AllGather is ~5x
faster per byte than a 4-rank ring.

Layout: activations live feature-major ("transposed", [D, tokens]) on chip so
every GEMM contracts along the partition axis with weights in natural layout.
The host transposes x in / y out. LayerNorm stats and softmax denominators
are partition-axis reductions done with ones-matmuls on the PE.

dtypes: weights are pre-cast to bf16 on the host; LN stats and residuals stay
fp32 (float32r for PE operands). All accumulation is fp32 in PSUM.

Scheduling notes: weight streams use [128, 1024] tiles with deep (bufs=8)
prefetch — shallow prefetch stalls the PE on DMA latency and re-throttles the
PE clock (HAM) to half rate. The attention exp is software-pipelined two
groups ahead of the PV/denominator matmuls that consume it.
"""
import sys

for _p in (
    "/root/.axon_site",
    "/root/.axon_site/_ro/trn_rl_repo",
    "/root/.axon_site/_ro/pypackages",
):
    if _p not in sys.path:
        sys.path.insert(0, _p)

import ml_dtypes
import numpy as np

import concourse.bacc as bacc
import concourse.tile as tile
import concourse.mybir as mybir
from concourse import bass_utils
from concourse.alu_op_type import AluOpType
from concourse.bass_interp import get_hw_module

B, S, D = 2, 2048, 2048
H, HD, FF = 16, 128, 8192
N_CORES = 8
TB = S // N_CORES  # 256 tokens of each batch per core
T = B * TB  # 512 tokens per core (256 b0 + 256 b1)
NCH = D // 128  # 16 feature chunks
FFCH = FF // 128  # 64 ff chunks
NKC = S // 128  # 16 key chunks per batch
F32 = mybir.dt.float32
F32R = mybir.dt.float32r
BF16 = mybir.dt.bfloat16
AF = mybir.ActivationFunctionType
OP = AluOpType
SM_SCALE = 1.0 / float(np.sqrt(HD))
WBUFS = 8  # weight-stream prefetch depth


def _layernorm(nc, tc, src, dst, g_s, b_s, ones_r, eps_t, name):
    """dst[:, c, :] = LN(src)[:, c, :] over the feature (partition x chunk)
    axis. src is f32r [128, NCH, T]; dst is bf16 [128, NCH, T]."""
    with (
        tc.tile_pool(name=f"{name}_p", bufs=1) as lp,
        tc.tile_pool(name=f"{name}_s", bufs=3) as ls,
        tc.tile_pool(name=f"{name}_ps", bufs=1, space="PSUM") as lps,
    ):
        mu_ps = lps.tile([128, T], F32, tag="mu")
        for c in range(NCH):
            nc.tensor.matmul(
                mu_ps[:], ones_r[:], src[:, c, :],
                start=(c == 0), stop=(c == NCH - 1),
            )
        mu = lp.tile([128, T], F32)
        nc.scalar.activation(mu[:], mu_ps[:], AF.Copy, scale=1.0 / D)

        xc = lp.tile([128, NCH, T], F32)
        var_ps = lps.tile([128, T], F32, tag="var")
        for c in range(NCH):
            nc.vector.tensor_tensor(
                xc[:, c, :], src[:, c, :].bitcast(F32), mu[:], OP.subtract
            )
            sq = ls.tile([128, T], F32R, tag="sq")
            nc.scalar.activation(sq[:], xc[:, c, :], AF.Square)
            nc.tensor.matmul(
                var_ps[:], ones_r[:], sq[:],
                start=(c == 0), stop=(c == NCH - 1),
            )
        sd = lp.tile([128, T], F32)
        nc.scalar.activation(sd[:], var_ps[:], AF.Sqrt, bias=eps_t[:], scale=1.0 / D)
        rsq = lp.tile([128, T], F32)
        nc.vector.reciprocal_approx_fast(rsq[:], sd[:])

        for c in range(NCH):
            tmp = ls.tile([128, T], F32, tag="lnt")
            nc.vector.tensor_tensor(tmp[:], xc[:, c, :], rsq[:], OP.mult)
            nc.vector.tensor_scalar(
                dst[:, c, :], tmp[:],
                g_s[:, c:c + 1], b_s[:, c:c + 1],
                OP.mult, OP.add,
            )


def build():
    nc = bacc.Bacc("TRN2", target_bir_lowering=False, debug=False,
                   num_devices=N_CORES)

    xT_d = nc.dram_tensor("xT", [D, T], F32, kind="ExternalInput")
    wq_d = nc.dram_tensor("wq16", [D, D], BF16, kind="ExternalInput")
    wk_d = nc.dram_tensor("wk16", [D, D], BF16, kind="ExternalInput")
    wv_d = nc.dram_tensor("wv16", [D, D], BF16, kind="ExternalInput")
    wo_d = nc.dram_tensor("wo16", [D, D], BF16, kind="ExternalInput")
    w1_d = nc.dram_tensor("w116", [D, FF], BF16, kind="ExternalInput")
    w2_d = nc.dram_tensor("w216", [FF, D], BF16, kind="ExternalInput")
    b1_d = nc.dram_tensor("b1r", [128, FFCH], F32, kind="ExternalInput")
    b2_d = nc.dram_tensor("b2r", [128, NCH], F32, kind="ExternalInput")
    g1_d = nc.dram_tensor("g1r", [128, NCH], F32, kind="ExternalInput")
    be1_d = nc.dram_tensor("be1r", [128, NCH], F32, kind="ExternalInput")
    g2_d = nc.dram_tensor("g2r", [128, NCH], F32, kind="ExternalInput")
    be2_d = nc.dram_tensor("be2r", [128, NCH], F32, kind="ExternalInput")
    yT_d = nc.dram_tensor("yT", [D, T], F32, kind="ExternalOutput")

    with tile.TileContext(nc) as tc:
        with (
            tc.tile_pool(name="cst", bufs=1) as cst,
            tc.tile_pool(name="resid", bufs=1) as resid,
            tc.tile_pool(name="dram", bufs=1, space="DRAM") as dram,
        ):
            ones_r = cst.tile([128, 128], F32R)
            nc.vector.memset(ones_r[:].bitcast(F32), 1.0)
            ones16 = cst.tile([128, 128], BF16)
            nc.vector.memset(ones16[:], 1.0)
            eps_t = cst.tile([128, 1], F32)
            nc.vector.memset(eps_t[:], 1e-5)
            g1_s = cst.tile([128, NCH], F32)
            be1_s = cst.tile([128, NCH], F32)
            g2_s = cst.tile([128, NCH], F32)
            be2_s = cst.tile([128, NCH], F32)
            b1_s = cst.tile([128, FFCH], F32)
            b2_s = cst.tile([128, NCH], F32)
            nc.sync.dma_start(g1_s[:], g1_d.ap())
            nc.sync.dma_start(be1_s[:], be1_d.ap())
            nc.sync.dma_start(g2_s[:], g2_d.ap())
            nc.sync.dma_start(be2_s[:], be2_d.ap())
            nc.sync.dma_start(b1_s[:], b1_d.ap())
            nc.sync.dma_start(b2_s[:], b2_d.ap())

            xTs = resid.tile([128, NCH, T], F32R)
            for c in range(NCH):
                nc.sync.dma_start(
                    xTs[:, c, :],
                    xT_d.ap()[c * 128:(c + 1) * 128, :].bitcast(F32R),
                )
            x2Ts = resid.tile([128, NCH, T], F32R)

            kin = dram.tile([D, T], BF16)
            vin = dram.tile([T, D], BF16)
            kout = dram.tile([N_CORES * D, T], BF16, addr_space="Shared")
            vout = dram.tile([N_CORES * T, D], BF16, addr_space="Shared")

            with tc.tile_pool(name="attnres", bufs=1) as ares:
                qT = ares.tile([128, H, T], BF16)  # Q^T; per-head slot reused for O^T

                with tc.tile_pool(name="p1", bufs=1) as p1:
                    hT = p1.tile([128, NCH, T], BF16)
                    _layernorm(nc, tc, xTs, hT, g1_s, be1_s, ones_r, eps_t, "ln1")

                    with (
                        tc.tile_pool(name="qkvs", bufs=WBUFS) as qs,
                        tc.tile_pool(name="qkvstg", bufs=4) as stg,
                        tc.tile_pool(name="qkvps", bufs=1, space="PSUM") as qps,
                    ):
                        # K^T = wk.T @ h^T   (feeds the AllGather first)
                        for blk in range(2):
                            kps = qps.tile([128, 8, T], F32, tag="qkv")
                            for c in range(NCH):
                                wt = qs.tile([128, 1024], BF16, tag="w")
                                nc.sync.dma_start(
                                    wt[:],
                                    wk_d.ap()[c * 128:(c + 1) * 128,
                                              blk * 1024:(blk + 1) * 1024],
                                )
                                for q in range(8):
                                    nc.tensor.matmul(
                                        kps[:, q, :], wt[:, q * 128:(q + 1) * 128],
                                        hT[:, c, :],
                                        start=(c == 0), stop=(c == NCH - 1),
                                    )
                            for q in range(8):
                                ks = stg.tile([128, T], BF16, tag="kstg")
                                nc.scalar.activation(ks[:], kps[:, q, :], AF.Copy)
                                dk = blk * 8 + q
                                nc.sync.dma_start(
                                    kin[dk * 128:(dk + 1) * 128, :], ks[:]
                                )
                        # V = h @ wv  (natural layout: lhsT = h^T chunk)
                        for blk in range(2):
                            vps = qps.tile([128, 8, T], F32, tag="qkv")
                            for c in range(NCH):
                                wt = qs.tile([128, 1024], BF16, tag="w")
                                nc.sync.dma_start(
                                    wt[:],
                                    wv_d.ap()[c * 128:(c + 1) * 128,
                                              blk * 1024:(blk + 1) * 1024],
                                )
                                for sub in range(2):
                                    for t_ in range(4):
                                        nc.tensor.matmul(
                                            vps[:, sub * 4 + t_, :],
                                            hT[:, c, t_ * 128:(t_ + 1) * 128],
                                            wt[:, sub * 512:(sub + 1) * 512],
                                            start=(c == 0), stop=(c == NCH - 1),
                                        )
                            for sub in range(2):
                                for t_ in range(4):
                                    vs = stg.tile([128, 512], BF16, tag="vstg")
                                    nc.scalar.activation(
                                        vs[:], vps[:, sub * 4 + t_, :], AF.Copy
                                    )
                                    dv = blk * 2 + sub
                                    nc.sync.dma_start(
                                        vin[t_ * 128:(t_ + 1) * 128,
                                            dv * 512:(dv + 1) * 512],
                                        vs[:],
                                    )

                        nc.gpsimd.collective_compute(
                            "AllGather",
                            OP.bypass,
                            replica_groups=[list(range(N_CORES))],
                            ins=[kin.opt()],
                            outs=[kout.opt()],
                        )
                        nc.gpsimd.collective_compute(
                            "AllGather",
                            OP.bypass,
                            replica_groups=[list(range(N_CORES))],
                            ins=[vin.opt()],
                            outs=[vout.opt()],
                        )

                        # Q^T (overlaps the collectives)
                        for blk in range(2):
                            qph = qps.tile([128, 8, T], F32, tag="qkv")
                            for c in range(NCH):
                                wt = qs.tile([128, 1024], BF16, tag="w")
                                nc.sync.dma_start(
                                    wt[:],
                                    wq_d.ap()[c * 128:(c + 1) * 128,
                                              blk * 1024:(blk + 1) * 1024],
                                )
                                for q in range(8):
                                    nc.tensor.matmul(
                                        qph[:, q, :], wt[:, q * 128:(q + 1) * 128],
                                        hT[:, c, :],
                                        start=(c == 0), stop=(c == NCH - 1),
                                    )
                            for q in range(8):
                                nc.scalar.activation(
                                    qT[:, blk * 8 + q, :], qph[:, q, :], AF.Copy
                                )

                # attention: per head, keys of both batches (32 chunks of 128).
                # chunk groups of 4; exp pipelined 2 groups ahead of PV/den.
                # group g<4: batch-0 keys (queries cols 0:256);
                # group g>=4: batch-1 keys (queries cols 256:512).
                with (
                    tc.tile_pool(name="atts", bufs=2) as ats,
                    tc.tile_pool(name="attv", bufs=2) as atv,
                    tc.tile_pool(name="attes", bufs=4) as aes,
                    tc.tile_pool(name="attps", bufs=3, space="PSUM") as aps,
                    tc.tile_pool(name="attps2", bufs=1, space="PSUM") as aps2,
                ):
                    for h in range(H):
                        kT = ats.tile([128, N_CORES, T], BF16, tag="kT")
                        for r in range(N_CORES):
                            nc.sync.dma_start(
                                kT[:, r, :],
                                kout[r * D + h * 128: r * D + (h + 1) * 128, :],
                            )
                        # V rows for this head: [4096 keys, 128] as 32 chunks
                        vh = atv.tile([128, 2 * NKC, HD], BF16, tag="vh")
                        for m in range(2 * NKC):
                            nc.sync.dma_start(
                                vh[:, m, :],
                                vout[m * 128:(m + 1) * 128,
                                     h * 128:(h + 1) * 128],
                            )
                        pv_ps = aps2.tile([128, T], F32, tag="pv")
                        den_ps = aps2.tile([128, T], F32, tag="den")

                        def s_group(g, h=h, kT=kT):
                            # 4 score matmuls for key chunks g*4..g*4+3
                            bb = g // 4  # batch half
                            s_ps = aps.tile([128, 4, TB], F32, tag="s")
                            for i in range(4):
                                kb = (g % 4) * 4 + i  # batch-local key chunk
                                r, half = kb // 2, kb % 2
                                nc.tensor.matmul(
                                    s_ps[:, i, :],
                                    kT[:, r, bb * 256 + half * 128:
                                       bb * 256 + half * 128 + 128],
                                    qT[:, h, bb * 256:(bb + 1) * 256],
                                    start=True, stop=True,
                                )
                            exps = aes.tile([128, 4, TB], BF16, tag="exp")
                            nc.scalar.activation(
                                exps[:], s_ps[:], AF.Exp, scale=SM_SCALE
                            )
                            return exps

                        def pvden_group(g, exps, h=h, vh=vh, pv_ps=pv_ps,
                                        den_ps=den_ps):
                            bb = g // 4
                            qsl = slice(bb * 256, (bb + 1) * 256)
                            for i in range(4):
                                kb = (g % 4) * 4 + i
                                r, half = kb // 2, kb % 2
                                m = r * 4 + bb * 2 + half
                                first = (g % 4) == 0 and i == 0
                                last = (g % 4) == 3 and i == 3
                                nc.tensor.matmul(
                                    pv_ps[:, qsl], vh[:, m, :], exps[:, i, :],
                                    start=first, stop=last,
                                )
                                nc.tensor.matmul(
                                    den_ps[:, qsl], ones16[:], exps[:, i, :],
                                    start=first, stop=last,
                                )

                        pending = []
                        for g in range(8):
                            pending.append((g, s_group(g)))
                            if len(pending) > 2:
                                gg, ee = pending.pop(0)
                                pvden_group(gg, ee)
                        for gg, ee in pending:
                            pvden_group(gg, ee)

                        rec = aes.tile([128, T], F32, tag="rec")
                        nc.vector.reciprocal_approx_fast(rec[:], den_ps[:])
                        # overwrite Q^T slot with O^T (Q^T[h] is dead now)
                        nc.vector.tensor_tensor(
                            qT[:, h, :], pv_ps[:], rec[:], OP.mult
                        )

                # o_proj + residual -> x2T
                with (
                    tc.tile_pool(name="ops", bufs=WBUFS) as osp,
                    tc.tile_pool(name="opps", bufs=1, space="PSUM") as ops_ps,
                ):
                    for blk in range(2):
                        o_ps = ops_ps.tile([128, 8, T], F32, tag="o")
                        for h in range(H):
                            wt = osp.tile([128, 1024], BF16, tag="wo")
                            nc.sync.dma_start(
                                wt[:],
                                wo_d.ap()[h * 128:(h + 1) * 128,
                                          blk * 1024:(blk + 1) * 1024],
                            )
                            for q in range(8):
                                nc.tensor.matmul(
                                    o_ps[:, q, :], wt[:, q * 128:(q + 1) * 128],
                                    qT[:, h, :],
                                    start=(h == 0), stop=(h == H - 1),
                                )
                        for q in range(8):
                            dc = blk * 8 + q
                            nc.vector.tensor_tensor(
                                x2Ts[:, dc, :], o_ps[:, q, :],
                                xTs[:, dc, :].bitcast(F32), OP.add,
                            )

            # FFN
            with tc.tile_pool(name="ffnres", bufs=1) as fres:
                h2T = fres.tile([128, NCH, T], BF16)
                _layernorm(nc, tc, x2Ts, h2T, g2_s, be2_s, ones_r, eps_t, "ln2")

                with tc.tile_pool(name="gpool", bufs=1) as gp:
                    gres = gp.tile([128, FFCH, T], BF16)
                    with (
                        tc.tile_pool(name="fc1s", bufs=WBUFS) as fs1,
                        tc.tile_pool(name="fc1ps", bufs=1, space="PSUM") as f1ps,
                    ):
                        for fb in range(8):
                            a_ps = f1ps.tile([128, 8, T], F32, tag="a")
                            for c in range(NCH):
                                wt = fs1.tile([128, 1024], BF16, tag="w1")
                                nc.sync.dma_start(
                                    wt[:],
                                    w1_d.ap()[c * 128:(c + 1) * 128,
                                              fb * 1024:(fb + 1) * 1024],
                                )
                                for q in range(8):
                                    nc.tensor.matmul(
                                        a_ps[:, q, :], wt[:, q * 128:(q + 1) * 128],
                                        h2T[:, c, :],
                                        start=(c == 0), stop=(c == NCH - 1),
                                    )
                            for q in range(8):
                                ffc = fb * 8 + q
                                nc.scalar.activation(
                                    gres[:, ffc, :], a_ps[:, q, :], AF.Gelu,
                                    bias=b1_s[:, ffc:ffc + 1],
                                )
                    with (
                        tc.tile_pool(name="fc2s", bufs=WBUFS) as fs2,
                        tc.tile_pool(name="fco", bufs=3) as fo,
                        tc.tile_pool(name="fc2ps", bufs=1, space="PSUM") as f2ps,
                    ):
                        for db in range(2):
                            y_ps = f2ps.tile([128, 8, T], F32, tag="y")
                            for f in range(FFCH):
                                wt = fs2.tile([128, 1024], BF16, tag="w2")
                                nc.sync.dma_start(
                                    wt[:],
                                    w2_d.ap()[f * 128:(f + 1) * 128,
                                              db * 1024:(db + 1) * 1024],
                                )
                                for q in range(8):
                                    nc.tensor.matmul(
                                        y_ps[:, q, :], wt[:, q * 128:(q + 1) * 128],
                                        gres[:, f, :],
                                        start=(f == 0), stop=(f == FFCH - 1),
                                    )
                            for q in range(8):
                                dc = db * 8 + q
                                yt = fo.tile([128, T], F32, tag="yt")
                                nc.vector.scalar_tensor_tensor(
                                    yt[:], y_ps[:, q, :], b2_s[:, dc:dc + 1],
                                    x2Ts[:, dc, :].bitcast(F32),
                                    OP.add, OP.add,
                                )
                                nc.sync.dma_start(
                                    yT_d.ap()[dc * 128:(dc + 1) * 128, :], yt[:]
                                )

    nc.compile()
    return nc


_NC_CACHE = None


def _get_nc():
    global _NC_CACHE
    if _NC_CACHE is None:
        m = build()
        m.m = get_hw_module(m.m)
        _NC_CACHE = m
    return _NC_CACHE


def _make_in_maps(x, wq, wk, wv, wo, w1, b1, w2, b2, g1, be1, g2, be2):
    f = lambda a: np.ascontiguousarray(np.asarray(a, dtype=np.float32))
    f16 = lambda a: np.ascontiguousarray(
        np.asarray(a, dtype=np.float32).astype(ml_dtypes.bfloat16)
    )
    x = f(x)
    shared = {
        "wq16": f16(wq), "wk16": f16(wk), "wv16": f16(wv), "wo16": f16(wo),
        "w116": f16(w1), "w216": f16(w2),
        "b1r": np.ascontiguousarray(f(b1).reshape(FFCH, 128).T),
        "b2r": np.ascontiguousarray(f(b2).reshape(NCH, 128).T),
        "g1r": np.ascontiguousarray(f(g1).reshape(NCH, 128).T),
        "be1r": np.ascontiguousarray(f(be1).reshape(NCH, 128).T),
        "g2r": np.ascontiguousarray(f(g2).reshape(NCH, 128).T),
        "be2r": np.ascontiguousarray(f(be2).reshape(NCH, 128).T),
    }
    in_maps = []
    for c in range(N_CORES):
        t0 = c * TB
        xc = np.concatenate([x[0, t0:t0 + TB, :], x[1, t0:t0 + TB, :]], axis=0)
        m = dict(shared)
        m["xT"] = np.ascontiguousarray(xc.T)
        in_maps.append(m)
    return in_maps


def _assemble(results):
    y = np.empty((B, S, D), dtype=np.float32)
    for c in range(N_CORES):
        t0 = c * TB
        yt = results[c]["yT"]
        y[0, t0:t0 + TB, :] = yt[:, 0:TB].T
        y[1, t0:t0 + TB, :] = yt[:, TB:2 * TB].T
    return y


def run(inputs, trace=False, trace_cores=None):
    nc = _get_nc()
    in_maps = _make_in_maps(**inputs)
    res = bass_utils.run_bass_kernel_spmd(
        nc, in_maps, core_ids=list(range(N_CORES)),
        trace=trace, trace_cores=trace_cores,
    )
    return _assemble(res.results), res


def kernel(**inputs):
    y, _ = run(inputs, trace=False)
    return y
